# revision 8
# baseline (speedup 1.0000x reference)
"""Conv3d (k=3, pad=1) as shifted-window matmuls on 8 TRN2 NeuronCores.

Sharding: data-parallel over (batch B=2) x (T quarters of 8 output frames).
Each core computes out[b, :, t0:t0+8, :, :] from a host-padded input shard
xs[ci, 10, 130, 130] (conv zero-padding + t-halo baked in by the host).

Per-core formulation: output tile M=128 = (co=32, dt=2, dh=2) output
positions, contraction K=128 = (jt=4 t-window slots, jhg=2 h-parity, ci=16),
N=512 = (4 h-blocks x 128 w). The 3x3x3 kernel becomes 6 accumulating
matmuls (kw=3 x jhh=2) per PSUM tile, with all w/h shifts expressed as free-
dim AP offsets into one SBUF-resident tile.

x/weights/output travel as bf16 (halves DMA, PE runs bf16 at full rate,
fp32 PSUM accumulate keeps error ~1e-3); bias stays fp32. Input loads go on
the SP DGE queue, output stores on the Activation DGE queue so stores never
head-of-line-block the next tile's load.
"""

import sys

if "/opt/trn_rl_repo" not in sys.path:
    sys.path.insert(0, "/opt/trn_rl_repo")

import numpy as np
import ml_dtypes

BF16 = ml_dtypes.bfloat16

import concourse.bass as bass
import concourse.mybir as mybir
import concourse.tile as tile
from concourse.bass_utils import run_bass_kernel_spmd

B, C_IN, T, H, W = 2, 16, 32, 128, 128
C_OUT, KS = 32, 3
N_CORES = 8
TSH = T // 4          # output frames per core
NBT = TSH // 2        # bt tiles per core (2 output frames each)
HB = H // 2           # h blocks (dh=2)
NNB = HB // 4         # 512-wide n-blocks per bt tile (4 h-blocks x 128 w)


def _split_excess_waits(nc, limit=1):
    """This walrus build accepts at most ONE sync-wait command per
    instruction. Move excess waits onto same-engine single-wait NoOps placed
    immediately before the instruction (identical blocking semantics)."""
    uid = 0
    for f in nc.m.functions:
        for bb in f.blocks:
            out = []
            for inst in bb.instructions:
                si = inst.sync_info
                if si is not None and si.on_wait and len(si.on_wait) > limit:
                    waits = list(si.on_wait)
                    excess, keep = waits[:-limit], waits[-limit:]
                    for k in range(0, len(excess), limit):
                        nop = mybir.InstNoOp(
                            name=f"wait_split_{uid}", ins=[], outs=[],
                            sync_info=mybir.SyncInfo(
                                on_wait=list(excess[k:k + limit]), on_update=[]))
                        nop.engine = inst.engine
                        nc.register_instruction(nop)
                        uid += 1
                        out.append(nop)
                    si.on_wait = keep
                out.append(inst)
            bb.instructions[:] = out
    return nc


def _build_program(split=True, repeat=1):
    nc = bass.Bass()
    f32 = mybir.dt.float32
    bf16 = mybir.dt.bfloat16
    # Host pre-arranges the shard partition-major so every tile load/store is
    # ONE <=3-dim DMA: xs[f, jhg, ci, bh, w], out[bt, dt, dh, co, bh, w].
    xs = nc.dram_tensor("xs", [TSH + 2, 2, C_IN, HB + 1, W + 2], bf16,
                        kind="ExternalInput")
    wt = nc.dram_tensor("wt", [6, 128, 128], bf16, kind="ExternalInput")
    bi = nc.dram_tensor("bi", [128, 1], f32, kind="ExternalInput")
    out = nc.dram_tensor("out", [NBT, 2, 2, C_OUT, HB, W], bf16,
                         kind="ExternalOutput")

    with tile.TileContext(nc) as tc:
        with tc.tile_pool(name="wpool", bufs=1) as wpool, \
             tc.tile_pool(name="xpool", bufs=8) as xpool, \
             tc.tile_pool(name="opool", bufs=3) as opool, \
             tc.tile_pool(name="pspool", bufs=2, space="PSUM") as pspool:
            # weights/bias go on the Activation DGE queue so the first x
            # chunk streams on SP concurrently with them.
            w_sb = wpool.tile([128, 6, 128], bf16)
            nc.scalar.dma_start(out=w_sb[:, :, :],
                                in_=wt.rearrange("i p m -> p i m"))
            b_sb = wpool.tile([128, 1], f32)
            nc.scalar.dma_start(out=b_sb[:, :], in_=bi[:, :])

            import contextlib
            rep_ctx = (tc.For_i(0, repeat, 1,
                                hint_engines=(mybir.EngineType.PE,
                                              mybir.EngineType.SP,
                                              mybir.EngineType.DVE,
                                              mybir.EngineType.Activation))
                       if repeat > 1 else contextlib.nullcontext())
            with rep_ctx:
                body(nc, tc, xs, w_sb, b_sb, out, xpool, opool, pspool)
    if split:
        _split_excess_waits(nc)
    return nc


def body(nc, tc, xs, w_sb, b_sb, out, xpool, opool, pspool):
    f32 = mybir.dt.float32
    bf16 = mybir.dt.bfloat16
    for bt in range(NBT):
        # x tile split into 4 h-chunks of 17 lines (1-line overlap) so the
        # first matmul starts ~2us after the first chunk lands instead of
        # waiting for the whole 2.2MB tile.
        xcs = []
        for g in range(NNB // 4):
            xc = xpool.tile([128, 17, W + 2], bf16, name=f"xc{g}")
            src = xs[2 * bt:2 * bt + 4, :, :, 16 * g:16 * g + 17].rearrange(
                "f j c b w -> (f j c) b w")
            nc.sync.dma_start(out=xc[:, :, :], in_=src)
            xcs.append(xc)

        dst = out[bt].rearrange("dt dh co b w -> (dt dh co) b w")
        for g in range(NNB // 4):
            xc = xcs[g]
            pss = [pspool.tile([128, 4, W], f32, name=f"ps{j}")
                   for j in range(4)]
            for i in range(6):
                kw, jhh = divmod(i, 2)
                lhsT = w_sb[:, i, :]
                for j in range(4):
                    rhs = xc[:, 4 * j + jhh:4 * j + jhh + 4, kw:kw + W]
                    nc.tensor.matmul(pss[j][:, :, :], lhsT, rhs,
                                     start=(i == 0), stop=(i == 5))
            if bt == NBT - 1 and g == NNB // 4 - 1:
                # Tail: per-j add+store so the drain after the last matmul is
                # one small eviction instead of a 16-line one.
                for j in range(4):
                    otj = opool.tile([128, 4, W], bf16, name=f"otj{j}")
                    nc.vector.tensor_scalar_add(
                        otj[:, :, :], pss[j][:, :, :], b_sb[:, 0:1])
                    nc.scalar.dma_start(
                        out=dst[:, 16 * g + 4 * j:16 * g + 4 * j + 4, :],
                        in_=otj[:, :, :])
            else:
                ot = opool.tile([128, 16, W], bf16, name="ot")
                for j in range(4):
                    nc.vector.tensor_scalar_add(
                        ot[:, 4 * j:4 * j + 4, :],
                        pss[j][:, :, :], b_sb[:, 0:1])
                nc.scalar.dma_start(out=dst[:, 16 * g:16 * g + 16, :],
                                    in_=ot[:, :, :])


_NC_CACHE = {}


def _get_nc(repeat=1):
    if repeat not in _NC_CACHE:
        _NC_CACHE[repeat] = _build_program(repeat=repeat)
    return _NC_CACHE[repeat]


def _pack_weights(weight):
    wt = np.zeros((6, 128, 128), np.float32)
    for kw in range(3):
        for jhh in range(2):
            i = kw * 2 + jhh
            for jt in range(4):
                for jhg in range(2):
                    jh = 2 * jhh + jhg
                    r0 = jt * 32 + jhg * 16
                    for dt in range(2):
                        kt = jt - dt
                        if not 0 <= kt < KS:
                            continue
                        for dh in range(2):
                            kh = jh - dh
                            if not 0 <= kh < KS:
                                continue
                            c0 = dt * 64 + dh * 32
                            wt[i, r0:r0 + 16, c0:c0 + 32] = \
                                weight[:, :, kt, kh, kw].T
    return wt


def _make_in_maps(x, weight, bias):
    x = np.asarray(x, dtype=np.float32)
    weight = np.asarray(weight, dtype=np.float32)
    bias = np.asarray(bias, dtype=np.float32)

    xp = np.zeros((B, C_IN, T + 2, H + 2, W + 2), np.float32)
    xp[:, :, 1:-1, 1:-1, 1:-1] = x
    wt = _pack_weights(weight).astype(BF16)
    bi = np.tile(bias, 4).reshape(128, 1).astype(np.float32)

    in_maps = []
    for c in range(N_CORES):
        b, q = divmod(c, 4)
        t0 = q * TSH
        sh = xp[b, :, t0:t0 + TSH + 2]                # [ci, f, 130, 130]
        sh = sh.reshape(C_IN, TSH + 2, HB + 1, 2, W + 2)
        sh = np.ascontiguousarray(sh.transpose(1, 3, 0, 2, 4)).astype(BF16)
        in_maps.append({"xs": sh, "wt": wt, "bi": bi})
    return in_maps


def _unshard(res):
    outp = np.empty((B, C_OUT, T, H, W), np.float32)
    for c in range(N_CORES):
        b, q = divmod(c, 4)
        r = res.results[c]["out"].astype(np.float32)  # [bt, dt, dh, co, bh, w]
        r = r.transpose(3, 0, 1, 4, 2, 5).reshape(C_OUT, TSH, H, W)
        outp[b, :, q * TSH:(q + 1) * TSH] = r
    return outp


def run(x, weight, bias, trace=False, repeat=1):
    in_maps = _make_in_maps(x, weight, bias)
    nc = _get_nc(repeat)
    res = run_bass_kernel_spmd(nc, in_maps, list(range(N_CORES)), trace=trace)
    return _unshard(res), res


def kernel(x, weight, bias):
    outp, _ = run(x, weight, bias, trace=False)
    return outp


# revision 15
# speedup vs baseline: 1.4845x; 1.4845x over previous
"""Conv3d (k=3, pad=1) as shifted-window matmuls on 8 TRN2 NeuronCores.

Sharding: data-parallel over (batch B=2) x (T quarters of 8 output frames).
Each core computes out[b, :, t0:t0+8, :, :] from a host-padded input shard
xs[ci, 10, 130, 130] (conv zero-padding + t-halo baked in by the host).

Per-core formulation: output tile M=128 = (co=32, dt=2, dh=2) output
positions, contraction K=128 = (jt=4 t-window slots, jhg=2 h-parity, ci=16),
N=512 = (4 h-blocks x 128 w). The 3x3x3 kernel becomes 6 accumulating
matmuls (kw=3 x jhh=2) per PSUM tile, with all w/h shifts expressed as free-
dim AP offsets into one SBUF-resident tile.

x/weights/output travel as bf16 (halves DMA, PE runs bf16 at full rate,
fp32 PSUM accumulate keeps error ~1e-3); bias stays fp32. Input loads go on
the SP DGE queue, output stores on the Activation DGE queue so stores never
head-of-line-block the next tile's load.
"""

import sys

if "/opt/trn_rl_repo" not in sys.path:
    sys.path.insert(0, "/opt/trn_rl_repo")

import numpy as np
import ml_dtypes

BF16 = ml_dtypes.bfloat16

import concourse.bass as bass
import concourse.mybir as mybir
import concourse.tile as tile
from concourse.bass_utils import run_bass_kernel_spmd

B, C_IN, T, H, W = 2, 16, 32, 128, 128
C_OUT, KS = 32, 3
N_CORES = 8
TSH = T // 4          # output frames per core
NBT = TSH // 2        # bt tiles per core (2 output frames each)
HB = H // 2           # h blocks (dh=2)
NNB = HB // 4         # 512-wide n-blocks per bt tile (4 h-blocks x 128 w)


def _split_excess_waits(nc, limit=1):
    """This walrus build accepts at most ONE sync-wait command per
    instruction. Move excess waits onto same-engine single-wait NoOps placed
    immediately before the instruction (identical blocking semantics)."""
    uid = 0
    for f in nc.m.functions:
        for bb in f.blocks:
            out = []
            for inst in bb.instructions:
                si = inst.sync_info
                if si is not None and si.on_wait and len(si.on_wait) > limit:
                    waits = list(si.on_wait)
                    excess, keep = waits[:-limit], waits[-limit:]
                    for k in range(0, len(excess), limit):
                        nop = mybir.InstNoOp(
                            name=f"wait_split_{uid}", ins=[], outs=[],
                            sync_info=mybir.SyncInfo(
                                on_wait=list(excess[k:k + limit]), on_update=[]))
                        nop.engine = inst.engine
                        nc.register_instruction(nop)
                        uid += 1
                        out.append(nop)
                    si.on_wait = keep
                out.append(inst)
            bb.instructions[:] = out
    return nc


def _build_program(split=True, repeat=1):
    nc = bass.Bass()
    f32 = mybir.dt.float32
    bf16 = mybir.dt.bfloat16
    # Host pre-arranges the shard partition-major so every tile load/store is
    # ONE <=3-dim DMA: xs[f, jhg, ci, bh, w], out[bt, dt, dh, co, bh, w].
    xs = nc.dram_tensor("xs", [TSH + 2, 2, C_IN, HB + 1, W + 2], bf16,
                        kind="ExternalInput")
    wt = nc.dram_tensor("wt", [6, 128, 128], bf16, kind="ExternalInput")
    bi = nc.dram_tensor("bi", [128, 1], f32, kind="ExternalInput")
    out = nc.dram_tensor("out", [NBT, 2, 2, C_OUT, HB, W], bf16,
                         kind="ExternalOutput")

    with tile.TileContext(nc) as tc:
        with tc.tile_pool(name="wpool", bufs=1) as wpool, \
             tc.tile_pool(name="xpool", bufs=8) as xpool, \
             tc.tile_pool(name="opool", bufs=3) as opool, \
             tc.tile_pool(name="pspool", bufs=2, space="PSUM") as pspool:
            # weights/bias go on the Activation DGE queue so the first x
            # chunk streams on SP concurrently with them.
            w_sb = wpool.tile([128, 6, 128], bf16)
            nc.scalar.dma_start(out=w_sb[:, :, :],
                                in_=wt.rearrange("i p m -> p i m"))
            b_sb = wpool.tile([128, 1], f32)
            nc.scalar.dma_start(out=b_sb[:, :], in_=bi[:, :])

            import contextlib
            rep_ctx = (tc.For_i(0, repeat, 1,
                                hint_engines=(mybir.EngineType.PE,
                                              mybir.EngineType.SP,
                                              mybir.EngineType.DVE,
                                              mybir.EngineType.Activation))
                       if repeat > 1 else contextlib.nullcontext())
            with rep_ctx:
                body(nc, tc, xs, w_sb, b_sb, out, xpool, opool, pspool)
    if split:
        _split_excess_waits(nc)
    return nc


def body(nc, tc, xs, w_sb, b_sb, out, xpool, opool, pspool):
    f32 = mybir.dt.float32
    bf16 = mybir.dt.bfloat16
    # x tiles load as 4 h-chunks of 17 lines (1-line overlap) per bt: on this
    # HW the per-queue DMA bandwidth (~115 GB/s effective) dominates, and
    # ~0.5MB chunks pipeline tightly under the PE stream, letting the first
    # matmul start ~2us after the first chunk lands.
    for bt in range(NBT):
        xcs = []
        for g in range(NNB // 4):
            xc = xpool.tile([128, 17, W + 2], bf16, name=f"xc{g}")
            src = xs[2 * bt:2 * bt + 4, :, :, 16 * g:16 * g + 17].rearrange(
                "f j c b w -> (f j c) b w")
            nc.sync.dma_start(out=xc[:, :, :], in_=src)
            xcs.append(xc)

        dst = out[bt].rearrange("dt dh co b w -> (dt dh co) b w")
        for g in range(NNB // 4):
            xc = xcs[g]
            pss = [pspool.tile([128, 4, W], f32, name=f"ps{j}")
                   for j in range(4)]
            for i in range(6):
                kw, jhh = divmod(i, 2)
                lhsT = w_sb[:, i, :]
                for j in range(4):
                    rhs = xc[:, 4 * j + jhh:4 * j + jhh + 4, kw:kw + W]
                    nc.tensor.matmul(pss[j][:, :, :], lhsT, rhs,
                                     start=(i == 0), stop=(i == 5))
            if bt == NBT - 1 and g == NNB // 4 - 1:
                # Tail: per-j add+store so the drain after the last matmul is
                # one small eviction instead of a 16-line one.
                for j in range(4):
                    otj = opool.tile([128, 4, W], bf16, name=f"otj{j}")
                    nc.vector.tensor_scalar_add(
                        otj[:, :, :], pss[j][:, :, :], b_sb[:, 0:1])
                    nc.scalar.dma_start(
                        out=dst[:, 16 * g + 4 * j:16 * g + 4 * j + 4, :],
                        in_=otj[:, :, :])
            else:
                ot = opool.tile([128, 16, W], bf16, name="ot")
                for j in range(4):
                    nc.vector.tensor_scalar_add(
                        ot[:, 4 * j:4 * j + 4, :],
                        pss[j][:, :, :], b_sb[:, 0:1])
                nc.scalar.dma_start(out=dst[:, 16 * g:16 * g + 16, :],
                                    in_=ot[:, :, :])


_NC_CACHE = {}


def _get_nc(repeat=1):
    if repeat not in _NC_CACHE:
        _NC_CACHE[repeat] = _build_program(repeat=repeat)
    return _NC_CACHE[repeat]


def _pack_weights(weight):
    wt = np.zeros((6, 128, 128), np.float32)
    for kw in range(3):
        for jhh in range(2):
            i = kw * 2 + jhh
            for jt in range(4):
                for jhg in range(2):
                    jh = 2 * jhh + jhg
                    r0 = jt * 32 + jhg * 16
                    for dt in range(2):
                        kt = jt - dt
                        if not 0 <= kt < KS:
                            continue
                        for dh in range(2):
                            kh = jh - dh
                            if not 0 <= kh < KS:
                                continue
                            c0 = dt * 64 + dh * 32
                            wt[i, r0:r0 + 16, c0:c0 + 32] = \
                                weight[:, :, kt, kh, kw].T
    return wt


def _make_in_maps(x, weight, bias):
    x = np.asarray(x, dtype=np.float32)
    weight = np.asarray(weight, dtype=np.float32)
    bias = np.asarray(bias, dtype=np.float32)

    xp = np.zeros((B, C_IN, T + 2, H + 2, W + 2), np.float32)
    xp[:, :, 1:-1, 1:-1, 1:-1] = x
    wt = _pack_weights(weight).astype(BF16)
    bi = np.tile(bias, 4).reshape(128, 1).astype(np.float32)

    in_maps = []
    for c in range(N_CORES):
        b, q = divmod(c, 4)
        t0 = q * TSH
        sh = xp[b, :, t0:t0 + TSH + 2]                # [ci, f, 130, 130]
        sh = sh.reshape(C_IN, TSH + 2, HB + 1, 2, W + 2)
        sh = np.ascontiguousarray(sh.transpose(1, 3, 0, 2, 4)).astype(BF16)
        in_maps.append({"xs": sh, "wt": wt, "bi": bi})
    return in_maps


def _unshard(res):
    outp = np.empty((B, C_OUT, T, H, W), np.float32)
    for c in range(N_CORES):
        b, q = divmod(c, 4)
        r = res.results[c]["out"].astype(np.float32)  # [bt, dt, dh, co, bh, w]
        r = r.transpose(3, 0, 1, 4, 2, 5).reshape(C_OUT, TSH, H, W)
        outp[b, :, q * TSH:(q + 1) * TSH] = r
    return outp


def run(x, weight, bias, trace=False, repeat=1):
    in_maps = _make_in_maps(x, weight, bias)
    nc = _get_nc(repeat)
    res = run_bass_kernel_spmd(nc, in_maps, list(range(N_CORES)), trace=trace)
    return _unshard(res), res


def kernel(x, weight, bias):
    outp, _ = run(x, weight, bias, trace=False)
    return outp


# revision 19
# speedup vs baseline: 4.8459x; 3.2644x over previous
"""Conv3d (k=3, pad=1) as shifted-window matmuls on 8 TRN2 NeuronCores.

Sharding: data-parallel over (batch B=2) x (T quarters of 8 output frames).
Each core computes out[b, :, t0:t0+8, :, :] from a host-padded input shard
xs[ci, 10, 130, 130] (conv zero-padding + t-halo baked in by the host).

Per-core formulation: output tile M=128 = (co=32, dt=2, dh=2) output
positions, contraction K=128 = (jt=4 t-window slots, jhg=2 h-parity, ci=16),
N=512 = (4 h-blocks x 128 w). The 3x3x3 kernel becomes 6 accumulating
matmuls (kw=3 x jhh=2) per PSUM tile, with all w/h shifts expressed as free-
dim AP offsets into one SBUF-resident tile.

x/weights/output travel as bf16 (halves DMA, PE runs bf16 at full rate,
fp32 PSUM accumulate keeps error ~1e-3); bias stays fp32. Input loads go on
the SP DGE queue, output stores on the Activation DGE queue so stores never
head-of-line-block the next tile's load.
"""

import sys

if "/opt/trn_rl_repo" not in sys.path:
    sys.path.insert(0, "/opt/trn_rl_repo")

import numpy as np
import ml_dtypes

BF16 = ml_dtypes.bfloat16

import concourse.bass as bass
import concourse.mybir as mybir
import concourse.tile as tile
from concourse.bass_utils import run_bass_kernel_spmd

B, C_IN, T, H, W = 2, 16, 32, 128, 128
C_OUT, KS = 32, 3
N_CORES = 8
TSH = T // 4          # output frames per core
NBT = TSH // 2        # bt tiles per core (2 output frames each)
HB = H // 2           # h blocks (dh=2)
NNB = HB // 4         # 512-wide n-blocks per bt tile (4 h-blocks x 128 w)


def _split_excess_waits(nc, limit=1):
    """This walrus build accepts at most ONE sync-wait command per
    instruction. Move excess waits onto same-engine single-wait NoOps placed
    immediately before the instruction (identical blocking semantics)."""
    uid = 0
    for f in nc.m.functions:
        for bb in f.blocks:
            out = []
            for inst in bb.instructions:
                si = inst.sync_info
                if si is not None and si.on_wait and len(si.on_wait) > limit:
                    waits = list(si.on_wait)
                    excess, keep = waits[:-limit], waits[-limit:]
                    for k in range(0, len(excess), limit):
                        nop = mybir.InstNoOp(
                            name=f"wait_split_{uid}", ins=[], outs=[],
                            sync_info=mybir.SyncInfo(
                                on_wait=list(excess[k:k + limit]), on_update=[]))
                        nop.engine = inst.engine
                        nc.register_instruction(nop)
                        uid += 1
                        out.append(nop)
                    si.on_wait = keep
                out.append(inst)
            bb.instructions[:] = out
    return nc


def _build_program(split=True, repeat=1):
    nc = bass.Bass()
    f32 = mybir.dt.float32
    bf16 = mybir.dt.bfloat16
    # Host pre-arranges the shard partition-major so every tile load/store is
    # ONE <=3-dim DMA: xs[f, jhg, ci, bh, w], out[bt, dt, dh, co, bh, w].
    xs = nc.dram_tensor("xs", [TSH + 2, 2, C_IN, HB + 1, W + 2], bf16,
                        kind="ExternalInput")
    wt = nc.dram_tensor("wt", [6, 128, 128], bf16, kind="ExternalInput")
    bi = nc.dram_tensor("bi", [128, 1], f32, kind="ExternalInput")
    out = nc.dram_tensor("out", [NBT, 2, 2, C_OUT, HB, W], bf16,
                         kind="ExternalOutput")

    with tile.TileContext(nc) as tc:
        with tc.tile_pool(name="wpool", bufs=1) as wpool, \
             tc.tile_pool(name="xpool", bufs=8) as xpool, \
             tc.tile_pool(name="opool", bufs=3) as opool, \
             tc.tile_pool(name="pspool", bufs=2, space="PSUM") as pspool:
            # weights/bias go on the Activation DGE queue so the first x
            # chunk streams on SP concurrently with them.
            w_sb = wpool.tile([128, 6, 128], bf16)
            nc.scalar.dma_start(out=w_sb[:, :, :],
                                in_=wt.rearrange("i p m -> p i m"))
            b_sb = wpool.tile([128, 1], f32)
            nc.scalar.dma_start(out=b_sb[:, :], in_=bi[:, :])

            import contextlib
            rep_ctx = (tc.For_i(0, repeat, 1,
                                hint_engines=(mybir.EngineType.PE,
                                              mybir.EngineType.SP,
                                              mybir.EngineType.DVE,
                                              mybir.EngineType.Activation))
                       if repeat > 1 else contextlib.nullcontext())
            with rep_ctx:
                body(nc, tc, xs, w_sb, b_sb, out, xpool, opool, pspool)
    if split:
        _split_excess_waits(nc)
    return nc


def body(nc, tc, xs, w_sb, b_sb, out, xpool, opool, pspool):
    f32 = mybir.dt.float32
    bf16 = mybir.dt.bfloat16
    # x tiles load as 4 h-chunks of 17 lines (1-line overlap) per bt: on this
    # HW the per-queue DMA bandwidth (~115 GB/s effective) dominates, and
    # ~0.5MB chunks pipeline tightly under the PE stream, letting the first
    # matmul start ~2us after the first chunk lands.
    for bt in range(NBT):
        xcs = []
        for g in range(NNB // 4):
            xc = xpool.tile([128, 17, W + 2], bf16, name=f"xc{g}")
            src = xs[2 * bt:2 * bt + 4, :, :, 16 * g:16 * g + 17].rearrange(
                "f j c b w -> (f j c) b w")
            nc.sync.dma_start(out=xc[:, :, :], in_=src)
            xcs.append(xc)

        dst = out[bt].rearrange("dt dh co b w -> (dt dh co) b w")
        for g in range(NNB // 4):
            xc = xcs[g]
            pss = [pspool.tile([128, 4, W], f32, name=f"ps{j}")
                   for j in range(4)]
            for i in range(6):
                kw, jhh = divmod(i, 2)
                lhsT = w_sb[:, i, :]
                for j in range(4):
                    rhs = xc[:, 4 * j + jhh:4 * j + jhh + 4, kw:kw + W]
                    nc.tensor.matmul(pss[j][:, :, :], lhsT, rhs,
                                     start=(i == 0), stop=(i == 5))
            if bt == NBT - 1 and g == NNB // 4 - 1:
                # Tail: per-j add+store so the drain after the last matmul is
                # one small eviction instead of a 16-line one.
                for j in range(4):
                    otj = opool.tile([128, 4, W], bf16, name=f"otj{j}")
                    nc.vector.tensor_scalar_add(
                        otj[:, :, :], pss[j][:, :, :], b_sb[:, 0:1])
                    nc.scalar.dma_start(
                        out=dst[:, 16 * g + 4 * j:16 * g + 4 * j + 4, :],
                        in_=otj[:, :, :])
            else:
                ot = opool.tile([128, 16, W], bf16, name="ot")
                for j in range(4):
                    nc.vector.tensor_scalar_add(
                        ot[:, 4 * j:4 * j + 4, :],
                        pss[j][:, :, :], b_sb[:, 0:1])
                nc.scalar.dma_start(out=dst[:, 16 * g:16 * g + 16, :],
                                    in_=ot[:, :, :])


_NC_CACHE = {}


def _get_nc(repeat=1):
    if repeat not in _NC_CACHE:
        _NC_CACHE[repeat] = _build_program(repeat=repeat)
    return _NC_CACHE[repeat]


def _pack_weights(weight):
    wt = np.zeros((6, 128, 128), np.float32)
    for kw in range(3):
        for jhh in range(2):
            i = kw * 2 + jhh
            for jt in range(4):
                for jhg in range(2):
                    jh = 2 * jhh + jhg
                    r0 = jt * 32 + jhg * 16
                    for dt in range(2):
                        kt = jt - dt
                        if not 0 <= kt < KS:
                            continue
                        for dh in range(2):
                            kh = jh - dh
                            if not 0 <= kh < KS:
                                continue
                            c0 = dt * 64 + dh * 32
                            wt[i, r0:r0 + 16, c0:c0 + 32] = \
                                weight[:, :, kt, kh, kw].T
    return wt


def _make_in_maps(x, weight, bias):
    x = np.asarray(x, dtype=np.float32)
    weight = np.asarray(weight, dtype=np.float32)
    bias = np.asarray(bias, dtype=np.float32)

    xp = np.zeros((B, C_IN, T + 2, H + 2, W + 2), np.float32)
    xp[:, :, 1:-1, 1:-1, 1:-1] = x
    wt = _pack_weights(weight).astype(BF16)
    bi = np.tile(bias, 4).reshape(128, 1).astype(np.float32)

    in_maps = []
    for c in range(N_CORES):
        b, q = divmod(c, 4)
        t0 = q * TSH
        sh = xp[b, :, t0:t0 + TSH + 2]                # [ci, f, 130, 130]
        sh = sh.reshape(C_IN, TSH + 2, HB + 1, 2, W + 2)
        sh = np.ascontiguousarray(sh.transpose(1, 3, 0, 2, 4)).astype(BF16)
        in_maps.append({"xs": sh, "wt": wt, "bi": bi})
    return in_maps


def _unshard(res):
    outp = np.empty((B, C_OUT, T, H, W), np.float32)
    for c in range(N_CORES):
        b, q = divmod(c, 4)
        r = res.results[c]["out"].astype(np.float32)  # [bt, dt, dh, co, bh, w]
        r = r.transpose(3, 0, 1, 4, 2, 5).reshape(C_OUT, TSH, H, W)
        outp[b, :, q * TSH:(q + 1) * TSH] = r
    return outp


def run(x, weight, bias, trace=False, repeat=1):
    in_maps = _make_in_maps(x, weight, bias)
    nc = _get_nc(repeat)
    res = run_bass_kernel_spmd(nc, in_maps, list(range(N_CORES)), trace=trace)
    return _unshard(res), res


def kernel(x, weight, bias):
    outp, _ = run(x, weight, bias, trace=False)
    return outp
